# revision 9
# baseline (speedup 1.0000x reference)
"""GCN encoder kernel for 8 Trainium2 NeuronCores.

Strategy (v2)
-------------
out = relu(relu(A_hat @ x @ W0) @ W1), A_hat = D^-1/2 (A + I) D^-1/2.

- Host pre-scales x~ = dinv * x (bf16) so no per-edge norm is needed on
  device: psum accumulates sum_j x~[src_j]; the dst-side dinv factor is
  applied once at psum eviction.  Self-loops are ordinary slots.
- Destinations are dealt across cores/tiles by (lo,hi) degree rank so the
  compile-time per-tile step counts are tight across all 8 cores.
- Gathers are single-node 256 B rows with dma_gather(transpose=True)
  giving feature-major [128, nslots] columns.  int16 idx range forces two
  source halves (x~ rows < 25000 vs >= 25000, separate DRAM base views).
- Aggregation into psum [feat=128, 512 dsts] per 4-tile supertile:
  hi-half slots via identity matmuls (wide prefix steps, amortized LDW),
  lo-half slots via DVE tensor_tensor adds (narrow dense prefix steps,
  no LDW) -- splits the column stream across two engines.
- Eviction multiplies by dinv[dst] (host-replicated f32 [128, NDP]) on
  DVE, then the two dense layers run on the supertile directly
  (h0T is already feature-major); output written bf16.
"""

import os
import sys

for _p in ("/opt/trn_rl_repo", "/root/.axon_site/_ro/trn_rl_repo"):
    if os.path.isdir(_p) and _p not in sys.path:
        sys.path.insert(0, _p)

import numpy as np
import ml_dtypes
from contextlib import ExitStack

import concourse.bass as bass
import concourse.tile as tile
from concourse import bacc, mybir
from concourse.bass_utils import run_bass_kernel_spmd
from concourse.ap import AP

P = 128
NCORES = 8
NQ = int(os.environ.get("NQ", "4"))
CALLW = 512             # max idx per gather call (transpose gather crashes
                        # the exec unit above ~512; 512 verified on HW)
bf16 = mybir.dt.bfloat16
f32 = mybir.dt.float32
i16 = mybir.dt.int16
BF = ml_dtypes.bfloat16

NLO = 25000             # nodes < NLO are "lo" rows; rest are "hi"
ZPAD = 25000            # per-half zero-row index (row NLO / row 2*NLO+1)

LO_ON_DVE = os.environ.get("LO_ON_DVE", "1") == "1"


def _prep(x, W0, W1, edge_index):
    N, F = x.shape
    H = W0.shape[1]
    ND = (N + NCORES - 1) // NCORES
    NT = (ND + P - 1) // P
    NDP = NT * P
    SUP = 4                      # tiles per supertile
    NSUP = (NT + SUP - 1) // SUP

    row = np.asarray(edge_index[0], dtype=np.int64)
    col = np.asarray(edge_index[1], dtype=np.int64)
    # self-loops are ordinary slots
    loops = np.arange(N, dtype=np.int64)
    row2 = np.concatenate([row, loops])
    col2 = np.concatenate([col, loops])

    deg = np.bincount(col2, minlength=N).astype(np.float32)
    dinv = (1.0 / np.sqrt(deg)).astype(np.float32)

    # x~ rows with per-half zero pad rows
    xt = np.zeros((2 * NLO + 2, F), dtype=BF)
    xs = (np.asarray(x, dtype=np.float32) * dinv[:, None]).astype(BF)
    xt[:NLO] = xs[:NLO]
    xt[NLO + 1:2 * NLO + 1] = xs[NLO:]

    islo = row2 < NLO
    idx16 = np.where(islo, row2, row2 - NLO).astype(np.int64)

    lo_cnt = np.bincount(col2[islo], minlength=N)
    hi_cnt = np.bincount(col2[~islo], minlength=N)

    # deal dsts by (hi, lo) desc rank -> core r%8, pos r//8.  hi primary:
    # the matmul (hi) phase pays padding for within-tile hi spread, while
    # the DVE (lo) phase is dense-prefix and pad-free by construction.
    order = np.lexsort((-lo_cnt, -hi_cnt))
    rank = np.empty(N, dtype=np.int64)
    rank[order] = np.arange(N)
    core_of = rank % NCORES
    dpos = rank // NCORES                 # dealt pos 0..ND-1

    # global per-orig-tile hi maxes (tight across cores by construction)
    t_orig = dpos // P
    n_hi_orig = np.zeros(NT, dtype=np.int64)
    np.maximum.at(n_hi_orig, t_orig, hi_cnt)
    n_hi_orig = np.maximum(n_hi_orig, 1)

    # reorder tiles within each supertile by n_hi desc (global order)
    tile_slot_of_orig = np.empty(NT, dtype=np.int64)
    for s in range(NSUP):
        ts = np.arange(s * SUP, min((s + 1) * SUP, NT))
        ord_t = ts[np.argsort(-n_hi_orig[ts], kind="stable")]
        tile_slot_of_orig[ord_t] = np.arange(s * SUP, s * SUP + len(ts))
    n_hi_T = np.empty(NT, dtype=np.int64)
    n_hi_T[tile_slot_of_orig] = n_hi_orig

    # per-core final positions: within-tile sort by lo desc
    fpos = np.empty(N, dtype=np.int64)
    dst_of_fpos = np.full((NCORES, NDP), -1, dtype=np.int64)
    for c in range(NCORES):
        m = core_of == c
        dsts = np.nonzero(m)[0]
        tp = dpos[dsts] // P
        o = np.lexsort((-lo_cnt[dsts], tp))
        dsts_o = dsts[o]
        tslot = tile_slot_of_orig[dpos[dsts_o] // P]
        # within each orig tile, dsts_o sorted lo desc; position = offset in tile
        pit = np.zeros(len(dsts_o), dtype=np.int64)
        tb = np.searchsorted(tp[o], np.arange(NT))
        cnts = np.bincount(tp[o], minlength=NT)
        for t in range(NT):
            pit[tb[t]:tb[t] + cnts[t]] = np.arange(cnts[t])
        fp = tslot * P + pit
        fpos[dsts_o] = fp
        dst_of_fpos[c, fp] = dsts_o

    # per-tile-slot lo widths W[T, j] = max over cores count(lo_cnt > j)
    max_lo = int(lo_cnt.max())
    max_hi = int(hi_cnt.max())
    W_lo = np.zeros((NT, max_lo), dtype=np.int64)
    for c in range(NCORES):
        d = dst_of_fpos[c]
        lc = np.where(d >= 0, lo_cnt[np.maximum(d, 0)], 0).reshape(NT, P)
        for T in range(NT):
            cnt = np.bincount(np.minimum(lc[T], max_lo), minlength=max_lo + 1)
            w = np.cumsum(cnt[::-1])[::-1][1:]   # w[j] = #dsts with lo_cnt > j
            np.maximum(W_lo[T], w, out=W_lo[T])

    # per-core HI/LO slot matrices indexed by final position
    HI = np.full((NCORES, NDP, max(max_hi, 1)), ZPAD, dtype=np.int16)
    LO = np.full((NCORES, NDP, max(max_lo, 1)), ZPAD, dtype=np.int16)
    e_core = core_of[col2]
    e_fpos = fpos[col2]
    for c in range(NCORES):
        m = e_core == c
        ef, ei, el = e_fpos[m], idx16[m], islo[m]
        for half, M in ((True, LO), (False, HI)):
            mm = el == half
            f, iv = ef[mm], ei[mm]
            o = np.argsort(f, kind="stable")
            f, iv = f[o], iv[o]
            starts = np.searchsorted(f, np.arange(NDP))
            j = np.arange(len(f)) - starts[f]
            M[c, f, j] = iv.astype(np.int16)

    # build step structure + per-core slot streams
    # steps: list of (kind, sup, ps_off, width, nsteps share g columns...)
    calls = []    # dicts: half, nidx, idxoff, pieces[(kind, sup, ps_off, width, g_off, first, last_mm, evict)]
    streams = [[] for _ in range(NCORES)]   # per-core list of idx arrays
    icols = 0

    cur = None

    def new_call(half):
        nonlocal cur
        cur = dict(half=half, nidx=0, pieces=[], segs=[[] for _ in range(NCORES)])
        calls.append(cur)

    def add_step(half, sup, ps_off, width, slices, first=False):
        # slices: per-core [width] int16 arrays
        nonlocal cur
        if cur is None or cur["half"] != half or cur["nidx"] + width > CALLW:
            new_call(half)
        g_off = cur["nidx"]
        cur["pieces"].append([("hi" if half == "hi" else "lo"), sup, ps_off,
                              width, g_off, first, False, False])
        for c in range(NCORES):
            cur["segs"][c].append(slices[c])
        cur["nidx"] += width

    for s in range(NSUP):
        ts = list(range(s * SUP, min((s + 1) * SUP, NT)))
        ntile = len(ts)
        # hi phase: prefix steps
        nh = [int(n_hi_T[T]) for T in ts]
        for j in range(nh[0]):
            a = sum(1 for v in nh if v > j)
            width = a * P
            sl = [HI[c, ts[0] * P: ts[0] * P + width, j] for c in range(NCORES)]
            add_step("hi", s, 0, width, sl, first=(j == 0))
        # mark last hi (matmul) piece of this supertile
        cur["pieces"][-1][6] = True
        # lo phase: per tile dense prefix
        for k, T in enumerate(ts):
            for j in range(max_lo):
                w = int(W_lo[T, j])
                if w == 0:
                    break
                sl = [LO[c, T * P: T * P + w, j] for c in range(NCORES)]
                add_step("lo", s, k * P, w, sl)
        # evict after the last piece of this supertile
        cur["pieces"][-1][7] = True

    # finalize calls: pad nidx to %128 with ZPAD, build idx arrays
    call_meta = []
    idx_blocks = [[] for _ in range(NCORES)]
    for ci, cl in enumerate(calls):
        nidx = cl["nidx"]
        npad = (-nidx) % P
        nidxp = nidx + npad
        for c in range(NCORES):
            v = np.concatenate(cl["segs"][c] + ([np.full(npad, ZPAD, np.int16)] if npad else []))
            b = v.reshape(nidxp // 16, 16).T        # [16, nidxp/16]
            idx_blocks[c].append(np.tile(b, (8, 1)))
        call_meta.append(dict(half=cl["half"], nidx=nidxp, idxoff=icols,
                              pieces=cl["pieces"]))
        icols += nidxp // 16

    in_maps = []
    unshard = []
    for c in range(NCORES):
        idx_arr = np.concatenate(idx_blocks[c], axis=1)
        drep = np.zeros((P, NDP), dtype=np.float32)
        d = dst_of_fpos[c]
        valid = d >= 0
        drep[:, valid] = dinv[d[valid]][None, :]
        in_maps.append({
            "xt": xt,
            "idx": np.ascontiguousarray(idx_arr),
            "drep": drep,
            "ident": np.eye(P, dtype=BF),
            "w0": W0.astype(BF),
            "w1lo": W1[:128].astype(BF),
            "w1hi": W1[128:].astype(BF),
        })
        unshard.append(d)

    meta = dict(N=N, F=F, H=H, ND=ND, NT=NT, NDP=NDP, NSUP=NSUP, SUP=SUP,
                icols=icols, calls=call_meta)
    return in_maps, unshard, meta


def _build(meta):
    F, H = meta["F"], meta["H"]
    NT, NDP, NSUP, SUP = meta["NT"], meta["NDP"], meta["NSUP"], meta["SUP"]
    icols = meta["icols"]
    calls = meta["calls"]
    NROWS = 2 * NLO + 2

    nc = bacc.Bacc(None, target_bir_lowering=False, debug=False,
                   num_devices=NCORES, num_swdge_queues=NQ,
                   dynamic_dma_scratch_size=128 * 1024)
    xt_d = nc.declare_dram_parameter("xt", [NROWS, F], bf16, isOutput=False)
    idx_d = nc.declare_dram_parameter("idx", [P, icols], i16, isOutput=False)
    drep_d = nc.declare_dram_parameter("drep", [P, NDP], f32, isOutput=False)
    ident_d = nc.declare_dram_parameter("ident", [P, P], bf16, isOutput=False)
    w0_d = nc.declare_dram_parameter("w0", [F, H], bf16, isOutput=False)
    w1lo_d = nc.declare_dram_parameter("w1lo", [128, H], bf16, isOutput=False)
    w1hi_d = nc.declare_dram_parameter("w1hi", [H - 128, H], bf16, isOutput=False)
    out_d = nc.declare_dram_parameter("out", [H, NDP], bf16, isOutput=True)

    with tile.TileContext(nc) as tc, ExitStack() as ctx:
        cpool = ctx.enter_context(tc.tile_pool(name="const", bufs=1))
        gpool = ctx.enter_context(tc.tile_pool(name="g", bufs=8))
        hpool = ctx.enter_context(tc.tile_pool(name="h0T", bufs=2))
        h1p = ctx.enter_context(tc.tile_pool(name="h1", bufs=2))
        opool = ctx.enter_context(tc.tile_pool(name="o", bufs=2))
        ps_s = ctx.enter_context(tc.tile_pool(name="ps_s", bufs=2, space="PSUM"))
        ps_u = ctx.enter_context(tc.tile_pool(name="ps_u", bufs=1, space="PSUM"))
        ps_v = ctx.enter_context(tc.tile_pool(name="ps_v", bufs=1, space="PSUM"))

        ident = cpool.tile([P, P], bf16)
        nc.sync.dma_start(ident[:], ident_d[:])
        # idx: head (first 8 calls) loads first so gathers start early
        NHEAD = min(8, len(calls))
        ihead = sum(cl["nidx"] // 16 for cl in calls[:NHEAD])
        idx_sbh = cpool.tile([P, max(ihead, 16)], i16)
        nc.sync.dma_start(idx_sbh[:, :ihead], idx_d[:, :ihead])
        idx_sbt = cpool.tile([P, max(icols - ihead, 16)], i16)
        if icols > ihead:
            nc.sync.dma_start(idx_sbt[:, :icols - ihead], idx_d[:, ihead:])
        w0_sb = cpool.tile([F, H], bf16)
        nc.sync.dma_start(w0_sb[:], w0_d[:])
        w1lo_sb = cpool.tile([128, H], bf16)
        nc.sync.dma_start(w1lo_sb[:], w1lo_d[:])
        w1hi_sb = cpool.tile([H - 128, H], bf16)
        nc.sync.dma_start(w1hi_sb[:], w1hi_d[:])
        drep_sb = cpool.tile([P, NDP], f32)
        nc.sync.dma_start(drep_sb[:], drep_d[:])

        sup_w = [min(SUP, NT - s * SUP) * P for s in range(NSUP)]
        ps_of_sup = {}

        def phase2(s):
            w = sup_w[s]
            scol = s * SUP * P
            accp = ps_of_sup.pop(s)
            h0T = hpool.tile([P, SUP * P], bf16, tag="h0T")
            with nc.allow_low_precision("bf16 h0 evict"):
                nc.vector.tensor_tensor(
                    out=h0T[:, :w], in0=accp[:, :w],
                    in1=drep_sb[:, scol:scol + w], op=mybir.AluOpType.mult)
            u1 = ps_u.tile([P, SUP * P], f32, tag="u1")
            u2 = ps_u.tile([P, SUP * P], f32, tag="u2")
            nc.tensor.matmul(u1[:, :w], lhsT=w0_sb[:, 0:128], rhs=h0T[:, :w],
                             start=True, stop=True)
            nc.tensor.matmul(u2[:, :w], lhsT=w0_sb[:, 128:H], rhs=h0T[:, :w],
                             start=True, stop=True)
            h1a = h1p.tile([P, SUP * P], bf16, tag="h1a")
            h1b = h1p.tile([P, SUP * P], bf16, tag="h1b")
            nc.scalar.activation(h1a[:, :w], u1[:, :w], mybir.ActivationFunctionType.Relu)
            nc.scalar.activation(h1b[:, :w], u2[:, :w], mybir.ActivationFunctionType.Relu)
            v1 = ps_v.tile([P, SUP * P], f32, tag="v1")
            v2 = ps_v.tile([P, SUP * P], f32, tag="v2")
            nc.tensor.matmul(v1[:, :w], lhsT=w1lo_sb[:, 0:128], rhs=h1a[:, :w], start=True, stop=False)
            nc.tensor.matmul(v1[:, :w], lhsT=w1hi_sb[:, 0:128], rhs=h1b[:, :w], start=False, stop=True)
            nc.tensor.matmul(v2[:, :w], lhsT=w1lo_sb[:, 128:H], rhs=h1a[:, :w], start=True, stop=False)
            nc.tensor.matmul(v2[:, :w], lhsT=w1hi_sb[:, 128:H], rhs=h1b[:, :w], start=False, stop=True)
            o1 = opool.tile([P, SUP * P], bf16, tag="o1")
            o2 = opool.tile([P, SUP * P], bf16, tag="o2")
            nc.scalar.activation(o1[:, :w], v1[:, :w], mybir.ActivationFunctionType.Relu)
            nc.scalar.activation(o2[:, :w], v2[:, :w], mybir.ActivationFunctionType.Relu)
            nc.sync.dma_start(out_d[0:128, scol:scol + w], o1[:, :w])
            nc.sync.dma_start(out_d[128:H, scol:scol + w], o2[:, :w])

        kq = 0
        for ci, cl in enumerate(calls):
            nidx = cl["nidx"]
            g = gpool.tile([P, CALLW], bf16, tag="g")
            out_ap = AP(g[:].tensor, g[:].offset,
                        [g[:].ap[0], [nidx, 1], [1, nidx]])
            in_ap = xt_d[:] if cl["half"] == "lo" else xt_d[NLO + 1:]
            ioff = cl["idxoff"]
            ncol = nidx // 16
            if ci < NHEAD:
                idxs_ap = idx_sbh[:, ioff:ioff + ncol]
            else:
                idxs_ap = idx_sbt[:, ioff - ihead:ioff - ihead + ncol]
            nc.gpsimd.dma_gather(
                out_ap=out_ap, in_ap=in_ap, idxs_ap=idxs_ap,
                num_idxs=nidx, num_idxs_reg=nidx, elem_size=F,
                transpose=True, single_packet=True, queue_num=kq % NQ)
            kq += 1
            for kind, s, ps_off, width, g_off, first, last_mm, evict in cl["pieces"]:
                if first:
                    ps_of_sup[s] = ps_s.tile([P, SUP * P], f32, tag="acc",
                                             name=f"acc{s}")
                accp = ps_of_sup[s]
                if kind == "hi" or not LO_ON_DVE:
                    nc.tensor.matmul(
                        accp[:, ps_off:ps_off + width], lhsT=ident[:],
                        rhs=g[:, g_off:g_off + width],
                        start=first, stop=last_mm,
                        skip_group_check=True)
                else:
                    nc.vector.tensor_tensor(
                        out=accp[:, ps_off:ps_off + width],
                        in0=accp[:, ps_off:ps_off + width],
                        in1=g[:, g_off:g_off + width], op=mybir.AluOpType.add)
                if evict:
                    phase2(s)
    nc.compile()
    return nc


def _run(inputs, trace=False):
    x = np.asarray(inputs["x"])
    W0 = np.asarray(inputs["W0"])
    W1 = np.asarray(inputs["W1"])
    edge_index = np.asarray(inputs["edge_index"])
    in_maps, unshard, meta = _prep(x, W0, W1, edge_index)
    nc = _build(meta)
    res = run_bass_kernel_spmd(nc, in_maps, core_ids=list(range(NCORES)), trace=trace)
    N, H = meta["N"], meta["H"]
    h = np.empty((N, H), dtype=np.float32)
    for c in range(NCORES):
        o = np.asarray(res.results[c]["out"], dtype=np.float32)   # [H, NDP]
        d = unshard[c]
        m = d >= 0
        h[d[m]] = o.T[m]
    return h, res


def kernel(**inputs) -> np.ndarray:
    h, _ = _run(inputs, trace=False)
    return h
